# revision 26
# baseline (speedup 1.0000x reference)
"""DSNet Trainium2 kernel: data-parallel over 8 NeuronCores.

Math: the reference's sequential Dempster-Shafer combination over P=200
prototypes is reformulated per class as a linear recurrence on the ratio
r_c = mass_c / omega. With dinv = 1/(3*(1-s*U)) and sd = s*dinv the step is

    r'_c = A*r_c + B,   A = 1/3 + u_c*sd,   B = u_c*sd

(the 1/3 arises because (1-s*U)*dinv == 1/3 exactly), which maps onto the
DVE tensor_tensor_scan instruction with data1 == T = u_c*sd and
data0 == T + 1/3. The recurrence contracts by ~1/3 per step, so (a) only the
last K=32 of 200 prototypes affect the result at fp32 precision, and (b)
class/chunk segments can be chained in one scan without resetting state —
contamination decays by ~(1/3)^31. Validated vs float64 over the full batch:
max rel err 1.3e-6. Final output: out_c = (r_c + 0.1) / (sum_c r_c + 1).
"""
import sys
import numpy as np

for _p in ("/opt/trn_rl_repo", "/root/.axon_site/_ro/trn_rl_repo"):
    if _p not in sys.path:
        sys.path.insert(0, _p)

import concourse.bass as bass
import concourse.tile as tile
from concourse import bacc
from concourse import mybir
from concourse.bass_utils import run_bass_kernel_spmd

F = 128      # features
P = 200      # prototypes
C = 10       # classes
K = 32       # truncated scan window
SEG = C * K  # 320
N_CORES = 8
GROUP = 8    # chunks of 128 rows fused per iteration


def _host_prep(x, w, xi, eta, beta, n_cores=N_CORES):
    f32 = np.float32
    x = np.asarray(x, f32); w = np.asarray(w, f32)
    xi = np.asarray(xi, f32); eta = np.asarray(eta, f32)
    beta = np.asarray(beta, f32)
    B = x.shape[0]
    Bc = B // n_cores
    nchunk = Bc // 128

    gamma = (eta * eta)[0]
    alpha = (1.0 / (1.0 + np.exp(-xi)))[0]
    wsq = (w * w).sum(-1)
    bsq = beta * beta
    u = bsq / (bsq.sum(-1, keepdims=True) + f32(1e-8))
    U = u.sum(-1)

    wT2 = np.ascontiguousarray(w.T * (2.0 * gamma)[None, :]).astype(f32)
    ctab = (-gamma * wsq + np.log(alpha)).astype(f32)
    gneg = (-gamma).astype(f32)

    ut = u[P - K:]
    u320 = np.empty(SEG, f32)
    for c in range(C):
        u320[c * K:(c + 1) * K] = ut[:, c]
        u320[c * K] *= 3.0          # first step of each segment: omega not tripled
    UtK = U[P - K:].copy()

    def bc(v, n=128):
        return np.ascontiguousarray(np.broadcast_to(v[None, :], (n, v.shape[0])))

    biases = np.array([1e-4, 1.0 / 3.0, 1.0, 0.1, 3.0], f32)
    tabs = {
        "wT2": wT2,
        "gnegtab": bc(gneg), "ctab": bc(ctab),
        "UtK": bc(UtK), "u320": bc(u320),
        "biastab": bc(biases),
    }

    xTf = x.T
    sq = (x.astype(np.float64) ** 2).sum(-1).astype(f32)   # ||x||^2

    in_maps = []
    for i in range(n_cores):
        sl = slice(i * Bc, (i + 1) * Bc)
        m = dict(tabs)
        m["xT"] = np.ascontiguousarray(xTf[:, sl])
        # sqT[p, ic] = ||x_{core_base + ic*128 + p}||^2
        m["sqT"] = np.ascontiguousarray(sq[sl].reshape(nchunk, 128).T)
        in_maps.append(m)
    return in_maps, Bc


def _host_untile(res_out, Bc):
    # staging layout [128, niter, GROUP, C] -> rows ic*128+p
    niter = Bc // (128 * GROUP)
    r = np.asarray(res_out).reshape(128, niter, GROUP, C)
    return r.transpose(1, 2, 0, 3).reshape(Bc, C)


def build(Bc, group=GROUP):
    nchunk = Bc // 128
    niter = nchunk // group
    assert Bc % (128 * group) == 0
    dt = mybir.dt.float32
    nc = bacc.Bacc()

    xT = nc.declare_dram_parameter("xT", [F, Bc], dt, isOutput=False)
    sqT = nc.declare_dram_parameter("sqT", [128, nchunk], dt, isOutput=False)
    wT2 = nc.declare_dram_parameter("wT2", [F, P], dt, isOutput=False)
    gnegtab = nc.declare_dram_parameter("gnegtab", [128, P], dt, isOutput=False)
    ctab = nc.declare_dram_parameter("ctab", [128, P], dt, isOutput=False)
    UtK = nc.declare_dram_parameter("UtK", [128, K], dt, isOutput=False)
    u320 = nc.declare_dram_parameter("u320", [128, SEG], dt, isOutput=False)
    biastab = nc.declare_dram_parameter("biastab", [128, 5], dt, isOutput=False)
    out = nc.declare_dram_parameter("out", [128, niter * group * C], dt,
                                    isOutput=True)

    AL = mybir.AluOpType
    AF = mybir.ActivationFunctionType
    G = group

    def rep(t, apdims):
        a = t[:] if hasattr(t, 'tile_num') or not isinstance(t, bass.AP) else t
        return bass.AP(tensor=a.tensor, offset=a.offset, ap=[a.ap[0]] + apdims)

    with tile.TileContext(nc) as tc:
        with (
            tc.tile_pool(name="consts", bufs=1) as consts,
            tc.tile_pool(name="xin", bufs=2) as xin,
            tc.tile_pool(name="work", bufs=3) as work,
            tc.tile_pool(name="stage", bufs=1) as stage,
            tc.tile_pool(name="psum", bufs=1, space="PSUM") as psum,
        ):
            t_wT2 = consts.tile([F, P], dt)
            t_gneg = consts.tile([128, P], dt)
            t_ct = consts.tile([128, P], dt)
            t_UtK = consts.tile([128, K], dt)
            t_u320 = consts.tile([128, SEG], dt)
            t_sqT = consts.tile([128, nchunk], dt)
            t_bias = consts.tile([128, 5], dt)
            # first iteration's dependencies first
            nc.sync.dma_start(out=t_ct[:], in_=ctab[:, :])
            nc.sync.dma_start(out=t_wT2[:], in_=wT2[:, :])

            t_stage = stage.tile([128, niter, G, C], dt)

            # prefetch all x tiles up front
            xtiles = []
            for g in range(niter):
                t_x = xin.tile([F, G * 128], dt, tag=f"xmega{g}")
                nc.sync.dma_start(out=t_x[:],
                                  in_=xT[:, g * G * 128:(g + 1) * G * 128])
                xtiles.append(t_x)
                if g == 0:
                    nc.sync.dma_start(out=t_gneg[:], in_=gnegtab[:, :])
                    nc.sync.dma_start(out=t_sqT[:], in_=sqT[:, :])
                elif g == 1:
                    nc.sync.dma_start(out=t_UtK[:], in_=UtK[:, :])
                    nc.sync.dma_start(out=t_u320[:], in_=u320[:, :])
                    nc.sync.dma_start(out=t_bias[:], in_=biastab[:, :])

            t3s = [None, None]

            def head(g):
                t_x = xtiles[g]
                t_t3 = work.tile([128, G, P], dt, tag="t3")
                t3s[g % 2] = t_t3
                for ic in range(G):
                    mm = psum.tile([128, P], dt, tag=f"mm{ic}")
                    nc.tensor.matmul(mm[:], t_x[:, ic * 128:(ic + 1) * 128],
                                     t_wT2[:], start=True, stop=True)
                    # t3 = (-gamma)*||x||^2 + 2gamma*(x.w)
                    nc.vector.scalar_tensor_tensor(
                        out=t_t3[:, ic, :], in0=t_gneg[:],
                        scalar=t_sqT[:, g * G + ic:g * G + ic + 1],
                        in1=mm[:], op0=AL.mult, op1=AL.add)
                # t3 += ctab
                nc.gpsimd.tensor_add(t_t3[:], t_t3[:],
                                     rep(t_ct[:], [[0, G], [1, P]]))

            def tail(g):
                t_t3 = t3s[g % 2]
                t_mxt = work.tile([128, G], dt, tag="mxt")
                nc.vector.reduce_max(out=t_mxt[:], in_=t_t3[:],
                                     axis=mybir.AxisListType.X)
                t_mx = work.tile([128, G], dt, tag="mx")
                nc.scalar.activation(t_mx[:], t_mxt[:], AF.Exp)
                t_mxr = work.tile([128, G], dt, tag="mxr")
                nc.scalar.add(t_mxr[:], t_mx[:], t_bias[:, 0:1])
                nc.vector.reciprocal(t_mxr[:], t_mxr[:])

                t_et = work.tile([128, G, K], dt, tag="et")
                nc.scalar.activation(t_et[:], t_t3[:, :, P - K:P], AF.Exp)
                t_st = work.tile([128, G, K], dt, tag="st")
                nc.vector.tensor_mul(t_st[:], t_et[:],
                                     rep(t_mxr[:], [[1, G], [0, K]]))

                # dinv = 1/(3 - 3*st*U); sd = st*dinv
                t_q = work.tile([128, G, K], dt, tag="q")
                nc.vector.tensor_mul(t_q[:], t_st[:],
                                     rep(t_UtK[:], [[0, G], [1, K]]))
                nc.scalar.activation(t_q[:], t_q[:], AF.Identity,
                                     bias=t_bias[:, 4:5], scale=-3.0)
                t_dinv = work.tile([128, G, K], dt, tag="dinv")
                nc.vector.reciprocal(t_dinv[:], t_q[:])
                t_sd = work.tile([128, G, K], dt, tag="sd")
                nc.vector.tensor_mul(t_sd[:], t_st[:], t_dinv[:])

                # T = u320 (x) sd over classes;  A = T + 1/3;  scan
                # (split into halves to pipeline Pool -> ACT -> DVE chain)
                t_T = work.tile([128, G * SEG], dt, tag="T")
                t_A = work.tile([128, G * SEG], dt, tag="A")
                t_r = work.tile([128, G * SEG], dt, tag="r")
                GH = G // 2
                for h in range(2):
                    c0 = h * GH * SEG
                    sda = t_sd[:, h * GH:(h + 1) * GH, :]
                    nc.gpsimd.tensor_mul(
                        t_T[:, c0:c0 + GH * SEG],
                        rep(t_u320[:], [[0, GH], [K, C], [1, K]]),
                        bass.AP(tensor=sda.tensor, offset=sda.offset,
                                ap=[sda.ap[0], [K, GH], [0, C], [1, K]]))
                    nc.scalar.add(t_A[:, c0:c0 + GH * SEG],
                                  t_T[:, c0:c0 + GH * SEG], t_bias[:, 1:2])
                    nc.vector.tensor_tensor_scan(
                        out=t_r[:, c0:c0 + GH * SEG],
                        data0=t_A[:, c0:c0 + GH * SEG],
                        data1=t_T[:, c0:c0 + GH * SEG],
                        initial=0.0 if h == 0 else t_r[:, c0 - 1:c0],
                        op0=AL.mult, op1=AL.add)

                # finals: r_c at col (ic*SEG + c*K + K-1)
                ra = t_r[:, K - 1:]
                r_str = bass.AP(tensor=ra.tensor, offset=ra.offset,
                                ap=[ra.ap[0], [SEG, G], [K, C]])
                t_S = work.tile([128, G], dt, tag="S")
                nc.vector.reduce_sum(out=t_S[:], in_=r_str,
                                     axis=mybir.AxisListType.X)
                t_Sr = work.tile([128, G], dt, tag="Sr")
                nc.scalar.add(t_Sr[:], t_S[:], t_bias[:, 2:3])
                nc.vector.reciprocal(t_Sr[:], t_Sr[:])
                t_o1 = work.tile([128, G, C], dt, tag="o1")
                nc.scalar.add(t_o1[:], r_str, t_bias[:, 3:4])
                nc.vector.tensor_mul(t_stage[:, g, :, :], t_o1[:],
                                     rep(t_Sr[:], [[1, G], [0, C]]))

            for g in range(niter + 1):
                if g < niter:
                    head(g)
                if g >= 1:
                    tail(g - 1)

            nc.sync.dma_start(out=out[:, :], in_=t_stage[:])

    nc.compile()
    return nc


_CACHE = {}


def _get_program(Bc):
    if Bc not in _CACHE:
        _CACHE[Bc] = build(Bc)
    return _CACHE[Bc]


def kernel(x, w, xi, eta, beta, _trace=False):
    in_maps, Bc = _host_prep(x, w, xi, eta, beta)
    nc = _get_program(Bc)
    res = run_bass_kernel_spmd(nc, in_maps, list(range(N_CORES)), trace=_trace)
    out = np.concatenate([_host_untile(res.results[i]["out"], Bc)
                          for i in range(N_CORES)], axis=0)
    if _trace:
        return out.astype(np.float32), res
    return out.astype(np.float32)
